# revision 5
# baseline (speedup 1.0000x reference)
"""2-layer GCN (GCNConv -> ReLU -> GCNConv) on 8 Trainium2 NeuronCores.

Strategy (vertex partition by destination):
  - Nodes are split into 8 contiguous dst-slices (12500 true nodes, padded to
    12544 = 98 windows of 128). Core k computes the output rows of its slice.
  - Phase A: core k computes h = x_slice @ W1 on the TensorEngine, writes it
    row-major (bf16, 256B-strided rows) to DRAM, then an AllGather replicates
    the full h to every core.
  - Aggregation (both layers share the same graph structure / metadata):
      * non-self edges are bucketed on the host by (dst-window of 128 nodes,
        src-chunk of 25088 rows) and padded to multiples of 128 slots;
      * dma_gather pulls h[src] rows (256B bf16) from the local DRAM replica
        into SBUF, 128 edges per tile.  This is the kernel's bottleneck: the
        GPSIMD Q7 generates gather descriptors at ~10 ns/descriptor, so
        everything else is engineered to hide underneath it;
      * the VectorEngine builds a norm-weighted one-hot S[e, n] =
        norm_e * (dst_local_e == n) (bf16) in ONE dual-op tensor_scalar
        against a host-provided iota;
      * the TensorEngine computes msg.T @ S, accumulating the transposed
        window aggregate [64, 128] in fp32 PSUM -- a segment-sum with no
        scatter.  Self-loops take a descriptor-free path: the window's own
        h rows are DMAd sequentially and hit the same matmul with a
        diagonal one-hot.
  - Layer 1 epilogue: ReLU(agg + b1) on ScalarE, transpose back via PE, store
    h1 rows, AllGather, repeat aggregation for layer 2.
  - Layer 2 epilogue: (agg2 @ W2 + b2) using linearity  A_hat(h1 W2) =
    (A_hat h1) W2, transpose, write dense output rows.
"""

import os
import sys

import numpy as np

sys.path.insert(0, "/opt/trn_rl_repo")

import ml_dtypes
import concourse.bacc as bacc
import concourse.mybir as mybir
import concourse.tile as tile
from concourse.bass_utils import run_bass_kernel_spmd

F32 = mybir.dt.float32
BF16 = mybir.dt.bfloat16
I16 = mybir.dt.int16
NPBF = ml_dtypes.bfloat16

P = 8  # NeuronCores


class Cfg:
    def __init__(self, n_nodes, f_in=128, f_h=64, f_out=40, nchunk=None):
        assert n_nodes % P == 0
        self.N = n_nodes
        self.SLICE = n_nodes // P
        self.NW = -(-self.SLICE // 128)          # windows per core
        self.PSLICE = self.NW * 128              # padded slice rows
        self.NPAD = P * self.PSLICE              # padded global rows
        if nchunk is None:
            nchunk = max(1, -(-self.NPAD // 32512))
            while self.NPAD % nchunk:
                nchunk += 1
        self.NCHUNK = nchunk
        self.CHROWS = self.NPAD // self.NCHUNK   # rows per gather chunk
        assert self.CHROWS <= 32767, "chunk must fit int16 gather indices"
        for gw in (7, 8, 6, 5, 9, 4, 10, 3, 11, 2, 13, 14, 12, 1):
            if self.NW % gw == 0:
                self.GW = gw                     # windows per gather group
                break
        self.NG = self.NW // self.GW             # gather groups
        self.F_IN, self.F_H, self.F_OUT = f_in, f_h, f_out
        self.FP = 128                            # padded (bf16) feature row


def build_plan(cfg, src, dst, w_edge):
    """Bucket non-self edges by (core, dst-window, src-chunk); lay out
    per-core gather indices + per-slot (dst_local, weight) metadata with
    static capacities shared across cores."""
    NW, NC, GW, NG = cfg.NW, cfg.NCHUNK, cfg.GW, cfg.NG

    core = dst // cfg.SLICE
    loc = dst - core * cfg.SLICE
    wloc = loc // 128
    dstl = (loc % 128).astype(np.float32)
    gsrc = (src // cfg.SLICE) * cfg.PSLICE + (src % cfg.SLICE)
    chunk = gsrc // cfg.CHROWS
    cidx = (gsrc % cfg.CHROWS).astype(np.int16)

    bucket = (core * NW + wloc) * NC + chunk
    order = np.argsort(bucket, kind="stable")
    counts = np.bincount(bucket, minlength=P * NW * NC).reshape(P, NW, NC)

    CAP = -(-counts // 128)
    CAP = CAP.max(axis=0)                        # [NW, NC] tiles, shared

    toff = np.zeros((NW, NC), np.int64)          # tile offset within call
    CT = np.zeros((NG, NC), np.int64)            # tiles per (group, chunk)
    gtb = np.zeros((NG, NC), np.int64)           # global tile base of call
    run = 0
    for g in range(NG):
        for c in range(NC):
            gtb[g, c] = run
            t = 0
            for w in range(g * GW, (g + 1) * GW):
                toff[w, c] = t
                t += CAP[w, c]
            CT[g, c] = t
            run += t
    tot_tiles = run

    starts = np.zeros(P * NW * NC + 1, np.int64)
    np.cumsum(counts.ravel(), out=starts[1:])
    rank = np.arange(len(order), dtype=np.int64) - starts[bucket[order]]

    e_core = core[order]
    e_w = wloc[order]
    e_c = chunk[order]
    e_g = e_w // GW
    e_tile = toff[e_w, e_c] + rank // 128        # tile within call
    e_gt = gtb[e_g, e_c] + e_tile                # global tile (meta column)
    e_p = rank % 128

    # meta arrays have NW extra trailing columns for the self-loop tiles
    dstl_arr = np.zeros((P, 128, tot_tiles + NW), np.float32)
    wv_arr = np.zeros((P, 128, tot_tiles + NW), np.float32)
    dstl_arr[e_core, e_p, e_gt] = dstl[order]
    wv_arr[e_core, e_p, e_gt] = w_edge[order]

    s_call = e_tile * 128 + e_p                  # slot within call
    icol = gtb[e_g, e_c] * 8 + s_call // 16
    ipart = s_call % 16
    idx_arr = np.zeros((P, 16, tot_tiles * 8), np.int16)
    idx_arr[e_core, ipart, icol] = cidx[order]
    idx_arr = np.tile(idx_arr, (1, 8, 1))        # replicate across partitions

    tables = dict(CAP=CAP, CT=CT, gtb=gtb, toff=toff, tot_tiles=int(tot_tiles))
    return tables, idx_arr, dstl_arr, wv_arr


def build_program(cfg, tables):
    NW, NC, GW, NG = cfg.NW, cfg.NCHUNK, cfg.GW, cfg.NG
    FH, FOUT, PS, FP = cfg.F_H, cfg.F_OUT, cfg.PSLICE, cfg.FP
    CAP, CT, gtb, toff = tables["CAP"], tables["CT"], tables["gtb"], tables["toff"]
    TT = tables["tot_tiles"]
    CTMAX = [int(CT[:, c].max()) for c in range(NC)]

    nc = bacc.Bacc("TRN2", num_devices=P, debug=False)

    x_d = nc.dram_tensor("x", [PS, 128], F32, kind="ExternalInput")
    W1_d = nc.dram_tensor("W1", [128, FH], F32, kind="ExternalInput")
    W2_d = nc.dram_tensor("W2", [FH, FOUT], F32, kind="ExternalInput")
    b1_d = nc.dram_tensor("b1", [FH, 1], F32, kind="ExternalInput")
    b2_d = nc.dram_tensor("b2", [FOUT, 1], F32, kind="ExternalInput")
    io_d = nc.dram_tensor("iota", [128, 128], BF16, kind="ExternalInput")
    id_d = nc.dram_tensor("ident", [128, 128], F32, kind="ExternalInput")
    ix_d = nc.dram_tensor("idx", [128, TT * 8], I16, kind="ExternalInput")
    dl_d = nc.dram_tensor("dstl", [128, TT + NW], F32, kind="ExternalInput")
    wv_d = nc.dram_tensor("wv", [128, TT + NW], F32, kind="ExternalInput")
    out_d = nc.dram_tensor("out", [PS, FOUT], F32, kind="ExternalOutput")

    rg = [list(range(P))]

    with tile.TileContext(nc) as tc:
        with (
            tc.tile_pool(name="const", bufs=1) as cpool,
            tc.tile_pool(name="meta", bufs=1) as mpool,
            tc.tile_pool(name="hown", bufs=1) as hpool,
            tc.tile_pool(name="gather", bufs=2) as gpool,
            tc.tile_pool(name="gidx", bufs=2) as ixpool,
            tc.tile_pool(name="xw", bufs=3) as xwpool,
            tc.tile_pool(name="ws", bufs=4) as wpool,
            tc.tile_pool(name="sbuild", bufs=6) as spool,
            tc.tile_pool(name="psA", bufs=3, space="PSUM") as psA,
            tc.tile_pool(name="psB", bufs=2, space="PSUM") as psB,
            tc.tile_pool(name="dram", bufs=1, space="DRAM") as dpool,
        ):
            W1s = cpool.tile([128, FH], F32, tag="W1")
            nc.sync.dma_start(W1s[:], W1_d[:])
            W2s = cpool.tile([FH, FOUT], F32, tag="W2")
            nc.sync.dma_start(W2s[:], W2_d[:])
            b1s = cpool.tile([FH, 1], F32, tag="b1")
            nc.sync.dma_start(b1s[:], b1_d[:])
            b2s = cpool.tile([FOUT, 1], F32, tag="b2")
            nc.sync.dma_start(b2s[:], b2_d[:])
            ios = cpool.tile([128, 128], BF16, tag="iota")
            nc.sync.dma_start(ios[:], io_d[:])
            ids = cpool.tile([128, 128], F32, tag="ident")
            nc.sync.dma_start(ids[:], id_d[:])
            dls = mpool.tile([128, TT + NW], F32, tag="dstl")
            nc.sync.dma_start(dls[:], dl_d[:])
            wvs = mpool.tile([128, TT + NW], F32, tag="wv")
            nc.sync.dma_start(wvs[:], wv_d[:])

            # bf16 h tensors with 256B row stride (only cols 0:FH are real)
            h_loc = dpool.tile([PS, FP], BF16, tag="h_loc")
            h1_loc = dpool.tile([PS, FP], BF16, tag="h1_loc")
            h_full = dpool.tile([cfg.NPAD, FP], BF16, tag="h_full")
            h1_full = dpool.tile([cfg.NPAD, FP], BF16, tag="h1_full")

            h_loc_v = h_loc[:].rearrange("(w p) f -> p w f", p=128)
            h1_loc_v = h1_loc[:].rearrange("(w p) f -> p w f", p=128)
            x_v = x_d[:].rearrange("(w p) f -> p w f", p=128)
            out_v = out_d[:].rearrange("(w p) f -> p w f", p=128)

            # ---- Phase A: h = x @ W1, stored row-major bf16 ---------------
            for w in range(NW):
                xw = xwpool.tile([128, 128], F32, tag="xw")
                nc.sync.dma_start(xw[:], x_v[:, w, :])
                pxT = psB.tile([128, 128], F32, tag="row")
                nc.tensor.matmul(pxT[:], xw[:], ids[:], start=True, stop=True)
                xT = xwpool.tile([128, 128], F32, tag="xT")
                nc.scalar.copy(xT[:], pxT[:])
                pa = psA.tile([FH, 128], F32, tag="agg")
                nc.tensor.matmul(pa[:], W1s[:], xT[:], start=True, stop=True)
                hT = wpool.tile([FH, 128], F32, tag="hT")
                nc.scalar.copy(hT[:], pa[:])
                pb = psB.tile([128, 128], F32, tag="row")
                nc.tensor.matmul(pb[:, 0:FH], hT[:], ids[0:FH, 0:FH],
                                 start=True, stop=True)
                hr = wpool.tile([128, FH], BF16, tag="hrow")
                nc.scalar.copy(hr[:], pb[:, 0:FH])
                nc.sync.dma_start(h_loc_v[:, w, 0:FH], hr[:])

            nc.gpsimd.collective_compute(
                "AllGather", mybir.AluOpType.bypass, replica_groups=rg,
                ins=[h_loc[:].opt()], outs=[h_full[:].opt()],
            )

            # ---- shared aggregation layer --------------------------------
            def agg_layer(src_full, loc_v, last):
                # own-slice rows for the descriptor-free self-loop path
                hown = hpool.tile([128, NW, FH], BF16,
                                  tag="hown2" if last else "hown1")
                nc.sync.dma_start(hown[:], loc_v[:, :, 0:FH])
                for g in range(NG):
                    gts = []
                    for c in range(NC):
                        ct = int(CT[g, c])
                        gt = gpool.tile([128, CTMAX[c], FP], BF16, tag=f"g{c}")
                        gts.append(gt)
                        if ct == 0:
                            continue
                        ixt = ixpool.tile([128, CTMAX[c] * 8], I16, tag=f"i{c}")
                        nc.sync.dma_start(
                            ixt[:, : ct * 8],
                            ix_d[:, gtb[g, c] * 8:(gtb[g, c] + ct) * 8],
                        )
                        nc.gpsimd.dma_gather(
                            out_ap=gt[:, :ct, :],
                            in_ap=src_full[c * cfg.CHROWS:(c + 1) * cfg.CHROWS, :],
                            idxs_ap=ixt[:, : ct * 8],
                            num_idxs=ct * 128,
                            num_idxs_reg=ct * 128,
                            elem_size=FP,
                            single_packet=False,
                        )
                    for w in range(g * GW, (g + 1) * GW):
                        ntile = int(CAP[w].sum()) + 1
                        pw = psA.tile([FH, 128], F32, tag="agg")
                        ti = 0
                        for c in range(NC):
                            for t in range(int(CAP[w, c])):
                                col = int(gtb[g, c] + toff[w, c] + t)
                                S = spool.tile([128, 128], BF16, tag="S")
                                nc.vector.tensor_scalar(
                                    S[:], ios[:],
                                    dls[:, col:col + 1], wvs[:, col:col + 1],
                                    mybir.AluOpType.is_equal,
                                    mybir.AluOpType.mult,
                                )
                                nc.tensor.matmul(
                                    pw[:],
                                    gts[c][:, int(toff[w, c]) + t, 0:FH],
                                    S[:],
                                    start=(ti == 0), stop=(ti == ntile - 1),
                                )
                                ti += 1
                        # self-loop tile: diagonal one-hot over own rows
                        col = TT + w
                        S = spool.tile([128, 128], BF16, tag="S")
                        nc.vector.tensor_scalar(
                            S[:], ios[:],
                            dls[:, col:col + 1], wvs[:, col:col + 1],
                            mybir.AluOpType.is_equal,
                            mybir.AluOpType.mult,
                        )
                        nc.tensor.matmul(pw[:], hown[:, w, :], S[:],
                                         start=False, stop=True)
                        if not last:
                            # h1 = relu(agg + b1), back to row-major bf16
                            hT1 = wpool.tile([FH, 128], F32, tag="hT")
                            nc.scalar.activation(
                                hT1[:], pw[:],
                                mybir.ActivationFunctionType.Relu,
                                bias=b1s[:, 0:1],
                            )
                            pb = psB.tile([128, 128], F32, tag="row")
                            nc.tensor.matmul(pb[:, 0:FH], hT1[:],
                                             ids[0:FH, 0:FH],
                                             start=True, stop=True)
                            hr = wpool.tile([128, FH], BF16, tag="hrow")
                            nc.scalar.copy(hr[:], pb[:, 0:FH])
                            nc.sync.dma_start(h1_loc_v[:, w, 0:FH], hr[:])
                        else:
                            # out = agg2 @ W2 + b2, back to row-major f32
                            a2 = wpool.tile([FH, 128], F32, tag="hT")
                            nc.scalar.copy(a2[:], pw[:])
                            po = psA.tile([FH, 128], F32, tag="agg")
                            nc.tensor.matmul(po[0:FOUT, :], W2s[:], a2[:],
                                             start=True, stop=True)
                            o2T = wpool.tile([FOUT, 128], F32, tag="o2T")
                            nc.vector.tensor_scalar(
                                o2T[:], po[0:FOUT, :], b2s[:, 0:1], None,
                                mybir.AluOpType.add,
                            )
                            pf = psB.tile([128, 128], F32, tag="row")
                            nc.tensor.matmul(pf[:, 0:FOUT], o2T[:],
                                             ids[0:FOUT, 0:FOUT],
                                             start=True, stop=True)
                            orow = wpool.tile([128, FOUT], F32, tag="orow")
                            nc.scalar.copy(orow[:], pf[:, 0:FOUT])
                            nc.sync.dma_start(out_v[:, w, :], orow[:])

            agg_layer(h_full[:], h_loc_v, last=False)
            nc.gpsimd.collective_compute(
                "AllGather", mybir.AluOpType.bypass, replica_groups=rg,
                ins=[h1_loc[:].opt()], outs=[h1_full[:].opt()],
            )
            agg_layer(h1_full[:], h1_loc_v, last=True)

    nc.compile()
    return nc


def _prep(cfg, x, edge_index, W1, b1, W2, b2):
    src = np.asarray(edge_index[0], dtype=np.int64)
    dst = np.asarray(edge_index[1], dtype=np.int64)
    x = np.asarray(x, dtype=np.float32)
    N = cfg.N

    deg = (np.bincount(dst, minlength=N) + 1.0).astype(np.float32)
    dinv = (1.0 / np.sqrt(deg)).astype(np.float32)

    w_e = (dinv[src] * dinv[dst]).astype(np.float32)
    tables, idx_arr, dstl_arr, wv_arr = build_plan(cfg, src, dst, w_e)

    # self-loop meta (the NW trailing columns): diagonal one-hot weights
    NW = cfg.NW
    TT = tables["tot_tiles"]
    selfw = np.zeros((P, 128, NW), np.float32)
    d2 = (dinv * dinv).reshape(P, cfg.SLICE)
    for k in range(P):
        flat = np.zeros(cfg.PSLICE, np.float32)
        flat[: cfg.SLICE] = d2[k]
        selfw[k] = flat.reshape(NW, 128).T
    dstl_arr[:, :, TT:] = np.broadcast_to(
        np.arange(128, dtype=np.float32)[None, :, None], (P, 128, NW))
    wv_arr[:, :, TT:] = selfw

    # per-core x slices, zero-padded to PSLICE rows
    xs = np.zeros((P, cfg.PSLICE, cfg.F_IN), np.float32)
    xs[:, : cfg.SLICE] = x.reshape(P, cfg.SLICE, cfg.F_IN)

    iota = np.broadcast_to(np.arange(128, dtype=np.float32),
                           (128, 128)).astype(NPBF)
    ident = np.eye(128, dtype=np.float32)
    W1f = np.asarray(W1, np.float32)
    W2f = np.asarray(W2, np.float32)
    b1f = np.asarray(b1, np.float32).reshape(cfg.F_H, 1)
    b2f = np.asarray(b2, np.float32).reshape(cfg.F_OUT, 1)

    in_maps = []
    for k in range(P):
        in_maps.append({
            "x": xs[k], "W1": W1f, "W2": W2f, "b1": b1f, "b2": b2f,
            "iota": iota, "ident": ident,
            "idx": idx_arr[k], "dstl": dstl_arr[k], "wv": wv_arr[k],
        })
    return tables, in_maps


def _enable_tracing():
    """This container's antenv lacks axon_hooks; install the NTFF profile
    hook ourselves and stub out the S3 artifact upload."""
    import types
    import antenv
    import concourse.bass_utils as bu

    if "antenv.axon_hooks" not in sys.modules:
        from trn_agent_boot.trn_boot import _ntff_profile_via_ctypes

        hook = _ntff_profile_via_ctypes("/opt/axon/libaxon_pjrt.so")
        mod = types.ModuleType("antenv.axon_hooks")
        mod.get_axon_ntff_profile_hook = lambda: hook
        sys.modules["antenv.axon_hooks"] = mod
        antenv.axon_hooks = mod
    bu.upload_artifacts = lambda tmpdir: "local://" + tmpdir


def run(cfg, x, edge_index, W1, b1, W2, b2, trace=False):
    tables, in_maps = _prep(cfg, x, edge_index, W1, b1, W2, b2)
    nc = build_program(cfg, tables)
    if trace:
        try:
            _enable_tracing()
        except Exception as e:  # tracing is best-effort
            print("tracing unavailable:", e)
            trace = False
    res = run_bass_kernel_spmd(nc, in_maps, core_ids=list(range(P)), trace=trace)
    outs = [res.results[k]["out"][: cfg.SLICE] for k in range(P)]
    out = np.concatenate(outs, axis=0)
    return out, res


def kernel(x, edge_index, W1, b1, W2, b2):
    cfg = Cfg(n_nodes=100000, f_in=128, f_h=64, f_out=40)
    trace = bool(os.environ.get("BASS_TRACE"))
    out, res = run(cfg, x, edge_index, W1, b1, W2, b2, trace=trace)
    if res.exec_time_ns is not None:
        print(f"HW exec time: {res.exec_time_ns} ns")
    return out.astype(np.float32)


# revision 7
# speedup vs baseline: 1.2025x; 1.2025x over previous
"""2-layer GCN (GCNConv -> ReLU -> GCNConv) on 8 Trainium2 NeuronCores.

Strategy (vertex partition by destination):
  - Nodes are split into 8 contiguous dst-slices (12500 true nodes, padded to
    12544 = 98 windows of 128). Core k computes the output rows of its slice.
  - Phase A: core k computes h = x_slice @ W1 on the TensorEngine, writes it
    row-major (bf16, 256B-strided rows) to DRAM, then an AllGather replicates
    the full h to every core.
  - Aggregation (both layers share the same graph structure / metadata):
      * non-self edges are packed on the host by (dst-window of 128 nodes,
        src-chunk of 25088 rows) at max-over-cores edge-count capacity (no
        per-bucket tile rounding -- tiles may straddle window boundaries);
      * dma_gather pulls h[src] rows (256B bf16) from the local DRAM replica
        into SBUF, 128 edge-slots per tile.  This is the kernel's bottleneck:
        the GPSIMD Q7 generates gather descriptors at ~10 ns/descriptor, so
        everything else is engineered to hide underneath it;
      * the VectorEngine builds norm-weighted one-hots S[e, col, n] =
        w_e * (dstl[e, col] == n) for a whole (group, chunk) block in TWO
        wide tensor_tensor ops (broadcast APs).  A tile straddling two
        windows gets one meta column per window; out-of-window slots carry
        dstl = 999 and vanish in the compare;
      * the TensorEngine computes msg.T @ S per meta column, accumulating
        transposed window aggregates [64, 128] in an fp32 PSUM slab
        [64, GW, 128] (one slice per window of the group) -- a segment-sum
        with no scatter.  Self-loops take a descriptor-free path: the
        window's own rows are DMAd sequentially and hit the same matmul
        with a diagonal one-hot.
  - Layer 1 epilogue: ReLU(agg + b1) on ScalarE, transpose back via PE, store
    h1 rows, AllGather, repeat aggregation for layer 2.
  - Layer 2 epilogue: (agg2 @ W2 + b2) using linearity  A_hat(h1 W2) =
    (A_hat h1) W2, transpose, write dense output rows.
"""

import os
import sys

import numpy as np

sys.path.insert(0, "/opt/trn_rl_repo")

import ml_dtypes
import concourse.bacc as bacc
import concourse.mybir as mybir
import concourse.tile as tile
from concourse.bass_utils import run_bass_kernel_spmd

F32 = mybir.dt.float32
BF16 = mybir.dt.bfloat16
I16 = mybir.dt.int16
NPBF = ml_dtypes.bfloat16

P = 8           # NeuronCores
NOMATCH = 999.0  # dstl value that can never match iota 0..127


class Cfg:
    def __init__(self, n_nodes, f_in=128, f_h=64, f_out=40, nchunk=None):
        assert n_nodes % P == 0
        self.N = n_nodes
        self.SLICE = n_nodes // P
        self.NW = -(-self.SLICE // 128)          # windows per core
        self.PSLICE = self.NW * 128              # padded slice rows
        self.NPAD = P * self.PSLICE              # padded global rows
        if nchunk is None:
            nchunk = max(1, -(-self.NPAD // 32512))
            while self.NPAD % nchunk:
                nchunk += 1
        self.NCHUNK = nchunk
        self.CHROWS = self.NPAD // self.NCHUNK   # rows per gather chunk
        assert self.CHROWS <= 32767, "chunk must fit int16 gather indices"
        for gw in (7, 8, 6, 5, 9, 4, 10, 3, 11, 2, 13, 14, 12, 1):
            if self.NW % gw == 0:
                self.GW = gw                     # windows per gather group
                break
        self.NG = self.NW // self.GW             # gather groups
        self.F_IN, self.F_H, self.F_OUT = f_in, f_h, f_out
        self.FP = 128                            # padded (bf16) feature row


def build_plan(cfg, src, dst, w_edge):
    """Pack non-self edges by (core, dst-window, src-chunk) at shared
    (max-over-cores) edge-count capacities; emit per-core gather indices and
    per-(tile, window-segment) metadata columns."""
    NW, NC, GW, NG = cfg.NW, cfg.NCHUNK, cfg.GW, cfg.NG

    core = dst // cfg.SLICE
    loc = dst - core * cfg.SLICE
    wloc = loc // 128
    dstl = (loc % 128).astype(np.float32)
    gsrc = (src // cfg.SLICE) * cfg.PSLICE + (src % cfg.SLICE)
    chunk = gsrc // cfg.CHROWS
    cidx = (gsrc % cfg.CHROWS).astype(np.int16)

    bucket = (core * NW + wloc) * NC + chunk
    order = np.argsort(bucket, kind="stable")
    counts = np.bincount(bucket, minlength=P * NW * NC).reshape(P, NW, NC)
    cap = counts.max(axis=0)                     # [NW, NC] edge slots, shared

    # static call layout + meta-column enumeration
    off = np.zeros((NW, NC), np.int64)           # slot offset within call
    CT = np.zeros((NG, NC), np.int64)            # slots per call (x128)
    slotbase = np.zeros((NG, NC), np.int64)      # global slot base of call
    # per call: list of (col -> (tile, wi, w)) and per (w,c): col per tile
    calls = {}
    colmap = {}
    ncols = 0
    srun = 0
    col_list = []                                # (g, c, tile, w)
    for g in range(NG):
        for c in range(NC):
            slotbase[g, c] = srun
            t = 0
            spans = []
            for w in range(g * GW, (g + 1) * GW):
                off[w, c] = t
                spans.append((w, t, t + int(cap[w, c])))
                t += int(cap[w, c])
            ct = -(-t // 128) * 128
            CT[g, c] = ct
            srun += ct
            cols = []
            for tt in range(ct // 128):
                lo, hi = tt * 128, tt * 128 + 128
                for (w, a, b) in spans:
                    if a < hi and b > lo:
                        colmap[(w, c, tt)] = ncols
                        cols.append((tt, w))
                        col_list.append((g, c, tt, w))
                        ncols += 1
            calls[(g, c)] = cols
    tot_slots = srun
    tot_cols = ncols

    starts = np.zeros(P * NW * NC + 1, np.int64)
    np.cumsum(counts.ravel(), out=starts[1:])
    rank = np.arange(len(order), dtype=np.int64) - starts[bucket[order]]

    e_core = core[order]
    e_w = wloc[order]
    e_c = chunk[order]
    e_g = e_w // GW
    e_slot = off[e_w, e_c] + rank                # slot within call
    e_tile = e_slot // 128
    e_p = e_slot % 128
    cmap = np.full((NW, NC, int(cap.max()) // 128 + 2), -1, np.int64)
    for (w, c, tt), j in colmap.items():
        ft = off[w, c] // 128
        if 0 <= tt - ft < cmap.shape[2]:
            cmap[w, c, tt - ft] = j
    e_col = cmap[e_w, e_c, e_tile - off[e_w, e_c] // 128]
    assert (e_col >= 0).all()

    dstl_arr = np.full((P, 128, tot_cols + NW), NOMATCH, np.float32)
    wv_arr = np.zeros((P, 128, tot_cols + NW), np.float32)
    dstl_arr[e_core, e_p, e_col] = dstl[order]
    wv_arr[e_core, e_p, e_col] = w_edge[order]

    gslot = slotbase[e_g, e_c] + e_slot          # global slot
    icol = gslot // 16
    ipart = gslot % 16
    idx_arr = np.zeros((P, 16, tot_slots // 16), np.int16)
    idx_arr[e_core, ipart, icol] = cidx[order]
    idx_arr = np.tile(idx_arr, (1, 8, 1))        # replicate across partitions

    tables = dict(CT=CT, slotbase=slotbase, calls=calls,
                  tot_slots=int(tot_slots), tot_cols=int(tot_cols))
    return tables, idx_arr, dstl_arr, wv_arr


def build_program(cfg, tables):
    NW, NC, GW, NG = cfg.NW, cfg.NCHUNK, cfg.GW, cfg.NG
    FH, FOUT, PS, FP = cfg.F_H, cfg.F_OUT, cfg.PSLICE, cfg.FP
    CT, slotbase, calls = tables["CT"], tables["slotbase"], tables["calls"]
    TC = tables["tot_cols"]
    TS = tables["tot_slots"]
    NTMAX = [int(CT[:, c].max()) // 128 for c in range(NC)]
    # per-call col base (cols are enumerated call-major)
    colbase = {}
    run = 0
    for g in range(NG):
        for c in range(NC):
            colbase[(g, c)] = run
            run += len(calls[(g, c)])
    NCMAX = [max(len(calls[(g, c)]) for g in range(NG)) for c in range(NC)]

    nc = bacc.Bacc("TRN2", num_devices=P, debug=False)

    x_d = nc.dram_tensor("x", [PS, 128], F32, kind="ExternalInput")
    W1_d = nc.dram_tensor("W1", [128, FH], F32, kind="ExternalInput")
    W2_d = nc.dram_tensor("W2", [FH, FOUT], F32, kind="ExternalInput")
    b1_d = nc.dram_tensor("b1", [FH, 1], F32, kind="ExternalInput")
    b2_d = nc.dram_tensor("b2", [FOUT, 1], F32, kind="ExternalInput")
    io_d = nc.dram_tensor("iota", [128, 128], BF16, kind="ExternalInput")
    id_d = nc.dram_tensor("ident", [128, 128], F32, kind="ExternalInput")
    ix_d = nc.dram_tensor("idx", [128, TS // 16], I16, kind="ExternalInput")
    dl_d = nc.dram_tensor("dstl", [128, TC + NW], BF16, kind="ExternalInput")
    wv_d = nc.dram_tensor("wv", [128, TC + NW], BF16, kind="ExternalInput")
    out_d = nc.dram_tensor("out", [PS, FOUT], F32, kind="ExternalOutput")

    rg = [list(range(P))]

    with tile.TileContext(nc) as tc:
        with (
            tc.tile_pool(name="const", bufs=1) as cpool,
            tc.tile_pool(name="meta", bufs=1) as mpool,
            tc.tile_pool(name="hown", bufs=1) as hpool,
            tc.tile_pool(name="gather", bufs=2) as gpool,
            tc.tile_pool(name="gidx", bufs=2) as ixpool,
            tc.tile_pool(name="xw", bufs=3) as xwpool,
            tc.tile_pool(name="ws", bufs=4) as wpool,
            tc.tile_pool(name="sbuild", bufs=2) as spool,
            tc.tile_pool(name="sself", bufs=2) as sspool,
            tc.tile_pool(name="psA", bufs=2, space="PSUM") as psA,
            tc.tile_pool(name="psB", bufs=2, space="PSUM") as psB,
            tc.tile_pool(name="dram", bufs=1, space="DRAM") as dpool,
        ):
            W1s = cpool.tile([128, FH], F32, tag="W1")
            nc.sync.dma_start(W1s[:], W1_d[:])
            W2s = cpool.tile([FH, FOUT], F32, tag="W2")
            nc.sync.dma_start(W2s[:], W2_d[:])
            b1s = cpool.tile([FH, 1], F32, tag="b1")
            nc.sync.dma_start(b1s[:], b1_d[:])
            b2s = cpool.tile([FOUT, 1], F32, tag="b2")
            nc.sync.dma_start(b2s[:], b2_d[:])
            ios = cpool.tile([128, 128], BF16, tag="iota")
            nc.sync.dma_start(ios[:], io_d[:])
            ids = cpool.tile([128, 128], F32, tag="ident")
            nc.sync.dma_start(ids[:], id_d[:])
            dls = mpool.tile([128, TC + NW], BF16, tag="dstl")
            nc.sync.dma_start(dls[:], dl_d[:])
            wvs = mpool.tile([128, TC + NW], BF16, tag="wv")
            nc.sync.dma_start(wvs[:], wv_d[:])

            # bf16 h tensors with 256B row stride (only cols 0:FH are real)
            h_loc = dpool.tile([PS, FP], BF16, tag="h_loc")
            h1_loc = dpool.tile([PS, FP], BF16, tag="h1_loc")
            h_full = dpool.tile([cfg.NPAD, FP], BF16, tag="h_full")
            h1_full = dpool.tile([cfg.NPAD, FP], BF16, tag="h1_full")

            h_loc_v = h_loc[:].rearrange("(w p) f -> p w f", p=128)
            h1_loc_v = h1_loc[:].rearrange("(w p) f -> p w f", p=128)
            x_v = x_d[:].rearrange("(w p) f -> p w f", p=128)
            out_v = out_d[:].rearrange("(w p) f -> p w f", p=128)

            def build_S(pool, tag, ncols_t, meta_lo, ncols):
                """Two wide DVE ops -> S[:, j, n] = wv_j * (iota_n == dstl_j)."""
                S = pool.tile([128, ncols_t, 128], BF16, tag=tag)
                io_b = ios[:].unsqueeze(1).broadcast_to([128, ncols, 128])
                dl_b = dls[:, meta_lo:meta_lo + ncols].unsqueeze(2) \
                    .broadcast_to([128, ncols, 128])
                wv_b = wvs[:, meta_lo:meta_lo + ncols].unsqueeze(2) \
                    .broadcast_to([128, ncols, 128])
                nc.vector.tensor_tensor(S[:, :ncols, :], io_b, dl_b,
                                        mybir.AluOpType.is_equal)
                nc.vector.tensor_tensor(S[:, :ncols, :], S[:, :ncols, :], wv_b,
                                        mybir.AluOpType.mult)
                return S

            # ---- Phase A: h = x @ W1, stored row-major bf16 ---------------
            for w in range(NW):
                xw = xwpool.tile([128, 128], F32, tag="xw")
                nc.sync.dma_start(xw[:], x_v[:, w, :])
                pxT = psB.tile([128, 128], F32, tag="row")
                nc.tensor.matmul(pxT[:], xw[:], ids[:], start=True, stop=True)
                xT = xwpool.tile([128, 128], F32, tag="xT")
                nc.scalar.copy(xT[:], pxT[:])
                pa = psA.tile([FH, GW, 128], F32, tag="agg")
                nc.tensor.matmul(pa[:, 0, :], W1s[:], xT[:],
                                 start=True, stop=True)
                hT = wpool.tile([FH, 128], F32, tag="hT")
                nc.scalar.copy(hT[:], pa[:, 0, :])
                pb = psB.tile([128, 128], F32, tag="row")
                nc.tensor.matmul(pb[:, 0:FH], hT[:], ids[0:FH, 0:FH],
                                 start=True, stop=True)
                hr = wpool.tile([128, FH], BF16, tag="hrow")
                nc.scalar.copy(hr[:], pb[:, 0:FH])
                nc.sync.dma_start(h_loc_v[:, w, 0:FH], hr[:])

            nc.gpsimd.collective_compute(
                "AllGather", mybir.AluOpType.bypass, replica_groups=rg,
                ins=[h_loc[:].opt()], outs=[h_full[:].opt()],
            )

            # ---- shared aggregation layer --------------------------------
            def agg_layer(src_full, loc_v, last):
                hown = hpool.tile([128, NW, FH], BF16,
                                  tag="hown2" if last else "hown1")
                nc.sync.dma_start(hown[:], loc_v[:, :, 0:FH])
                for g in range(NG):
                    gts, Ss = [], []
                    for c in range(NC):
                        ct = int(CT[g, c])
                        nt = ct // 128
                        gt = gpool.tile([128, NTMAX[c], FP], BF16, tag=f"g{c}")
                        gts.append(gt)
                        ncol = len(calls[(g, c)])
                        if ct == 0:
                            Ss.append(None)
                            continue
                        ixt = ixpool.tile([128, NTMAX[c] * 8], I16, tag=f"i{c}")
                        nc.sync.dma_start(
                            ixt[:, : ct // 16],
                            ix_d[:, slotbase[g, c] // 16:
                                 (slotbase[g, c] + ct) // 16],
                        )
                        nc.gpsimd.dma_gather(
                            out_ap=gt[:, :nt, :],
                            in_ap=src_full[c * cfg.CHROWS:(c + 1) * cfg.CHROWS, :],
                            idxs_ap=ixt[:, : ct // 16],
                            num_idxs=ct,
                            num_idxs_reg=ct,
                            elem_size=FP,
                            single_packet=False,
                        )
                        Ss.append(build_S(spool, f"S{c}", NCMAX[c],
                                          colbase[(g, c)], ncol))
                    Sself = build_S(sspool, "Sself", GW, TC + g * GW, GW)
                    pw = psA.tile([FH, GW, 128], F32, tag="agg")
                    nc.vector.memset(pw[:], 0.0)
                    for c in range(NC):
                        for ci, (tt, w) in enumerate(calls[(g, c)]):
                            wi = w - g * GW
                            nc.tensor.matmul(
                                pw[:, wi, :],
                                gts[c][:, tt, 0:FH],
                                Ss[c][:, ci, :],
                                start=False, stop=False,
                                skip_group_check=True,
                            )
                    for wi in range(GW):
                        w = g * GW + wi
                        # self-loop tile: diagonal one-hot over own rows
                        nc.tensor.matmul(pw[:, wi, :], hown[:, w, :],
                                         Sself[:, wi, :],
                                         start=False, stop=True,
                                         skip_group_check=True)
                        if not last:
                            hT1 = wpool.tile([FH, 128], F32, tag="hT")
                            nc.scalar.activation(
                                hT1[:], pw[:, wi, :],
                                mybir.ActivationFunctionType.Relu,
                                bias=b1s[:, 0:1],
                            )
                            pb = psB.tile([128, 128], F32, tag="row")
                            nc.tensor.matmul(pb[:, 0:FH], hT1[:],
                                             ids[0:FH, 0:FH],
                                             start=True, stop=True)
                            hr = wpool.tile([128, FH], BF16, tag="hrow")
                            nc.scalar.copy(hr[:], pb[:, 0:FH])
                            nc.sync.dma_start(h1_loc_v[:, w, 0:FH], hr[:])
                        else:
                            a2 = wpool.tile([FH, 128], F32, tag="hT")
                            nc.scalar.copy(a2[:], pw[:, wi, :])
                            po = psB.tile([128, 128], F32, tag="row")
                            nc.tensor.matmul(po[0:FOUT, :], W2s[:], a2[:],
                                             start=True, stop=True)
                            o2T = wpool.tile([FOUT, 128], F32, tag="o2T")
                            nc.vector.tensor_scalar(
                                o2T[:], po[0:FOUT, :], b2s[:, 0:1], None,
                                mybir.AluOpType.add,
                            )
                            pf = psB.tile([128, 128], F32, tag="row")
                            nc.tensor.matmul(pf[:, 0:FOUT], o2T[:],
                                             ids[0:FOUT, 0:FOUT],
                                             start=True, stop=True)
                            orow = wpool.tile([128, FOUT], F32, tag="orow")
                            nc.scalar.copy(orow[:], pf[:, 0:FOUT])
                            nc.sync.dma_start(out_v[:, w, :], orow[:])

            agg_layer(h_full[:], h_loc_v, last=False)
            nc.gpsimd.collective_compute(
                "AllGather", mybir.AluOpType.bypass, replica_groups=rg,
                ins=[h1_loc[:].opt()], outs=[h1_full[:].opt()],
            )
            agg_layer(h1_full[:], h1_loc_v, last=True)

    nc.compile()
    return nc


def _prep(cfg, x, edge_index, W1, b1, W2, b2):
    src = np.asarray(edge_index[0], dtype=np.int64)
    dst = np.asarray(edge_index[1], dtype=np.int64)
    x = np.asarray(x, dtype=np.float32)
    N = cfg.N

    deg = (np.bincount(dst, minlength=N) + 1.0).astype(np.float32)
    dinv = (1.0 / np.sqrt(deg)).astype(np.float32)

    w_e = (dinv[src] * dinv[dst]).astype(np.float32)
    tables, idx_arr, dstl_arr, wv_arr = build_plan(cfg, src, dst, w_e)

    # self-loop meta (the NW trailing columns): diagonal one-hot weights
    NW = cfg.NW
    TC = tables["tot_cols"]
    selfw = np.zeros((P, 128, NW), np.float32)
    d2 = (dinv * dinv).reshape(P, cfg.SLICE)
    for k in range(P):
        flat = np.zeros(cfg.PSLICE, np.float32)
        flat[: cfg.SLICE] = d2[k]
        selfw[k] = flat.reshape(NW, 128).T
    dstl_arr[:, :, TC:] = np.broadcast_to(
        np.arange(128, dtype=np.float32)[None, :, None], (P, 128, NW))
    wv_arr[:, :, TC:] = selfw

    # per-core x slices, zero-padded to PSLICE rows
    xs = np.zeros((P, cfg.PSLICE, cfg.F_IN), np.float32)
    xs[:, : cfg.SLICE] = x.reshape(P, cfg.SLICE, cfg.F_IN)

    iota = np.broadcast_to(np.arange(128, dtype=np.float32),
                           (128, 128)).astype(NPBF)
    ident = np.eye(128, dtype=np.float32)
    W1f = np.asarray(W1, np.float32)
    W2f = np.asarray(W2, np.float32)
    b1f = np.asarray(b1, np.float32).reshape(cfg.F_H, 1)
    b2f = np.asarray(b2, np.float32).reshape(cfg.F_OUT, 1)

    in_maps = []
    for k in range(P):
        in_maps.append({
            "x": xs[k], "W1": W1f, "W2": W2f, "b1": b1f, "b2": b2f,
            "iota": iota, "ident": ident,
            "idx": idx_arr[k],
            "dstl": dstl_arr[k].astype(NPBF),
            "wv": wv_arr[k].astype(NPBF),
        })
    return tables, in_maps


def _enable_tracing():
    """This container's antenv lacks axon_hooks; install the NTFF profile
    hook ourselves and stub out the S3 artifact upload."""
    import types
    import antenv
    import concourse.bass_utils as bu

    if "antenv.axon_hooks" not in sys.modules:
        from trn_agent_boot.trn_boot import _ntff_profile_via_ctypes

        hook = _ntff_profile_via_ctypes("/opt/axon/libaxon_pjrt.so")
        mod = types.ModuleType("antenv.axon_hooks")
        mod.get_axon_ntff_profile_hook = lambda: hook
        sys.modules["antenv.axon_hooks"] = mod
        antenv.axon_hooks = mod
    bu.upload_artifacts = lambda tmpdir: "local://" + tmpdir


def run(cfg, x, edge_index, W1, b1, W2, b2, trace=False):
    tables, in_maps = _prep(cfg, x, edge_index, W1, b1, W2, b2)
    nc = build_program(cfg, tables)
    if trace:
        try:
            _enable_tracing()
        except Exception as e:  # tracing is best-effort
            print("tracing unavailable:", e)
            trace = False
    res = run_bass_kernel_spmd(nc, in_maps, core_ids=list(range(P)), trace=trace)
    outs = [res.results[k]["out"][: cfg.SLICE] for k in range(P)]
    out = np.concatenate(outs, axis=0)
    return out, res


def kernel(x, edge_index, W1, b1, W2, b2):
    cfg = Cfg(n_nodes=100000, f_in=128, f_h=64, f_out=40)
    trace = bool(os.environ.get("BASS_TRACE"))
    out, res = run(cfg, x, edge_index, W1, b1, W2, b2, trace=trace)
    if res.exec_time_ns is not None:
        print(f"HW exec time: {res.exec_time_ns} ns")
    return out.astype(np.float32)


# revision 8
# speedup vs baseline: 1.2890x; 1.0719x over previous
"""2-layer GCN (GCNConv -> ReLU -> GCNConv) on 8 Trainium2 NeuronCores.

Strategy (vertex partition by destination):
  - Nodes are split into 8 contiguous dst-slices (12500 true nodes, padded to
    12544 = 98 windows of 128). Core k computes the output rows of its slice.
  - Phase A: core k computes h = x_slice @ W1 on the TensorEngine, writes it
    row-major (bf16, 256B-strided rows) to DRAM, then an AllGather replicates
    the full h to every core.
  - Aggregation (both layers share the same graph structure / metadata):
      * non-self edges are packed on the host by (dst-window of 128 nodes,
        src-chunk of 25088 rows) at max-over-cores edge-count capacity (no
        per-bucket tile rounding -- tiles may straddle window boundaries);
      * dma_gather pulls h[src] rows (256B bf16) from the local DRAM replica
        into SBUF, 128 edge-slots per tile.  This is the kernel's bottleneck:
        the GPSIMD Q7 generates gather descriptors at ~10 ns/descriptor, so
        everything else is engineered to hide underneath it;
      * the VectorEngine builds norm-weighted one-hots S[e, col, n] =
        w_e * (dstl[e, col] == n) for a whole (group, chunk) block in TWO
        wide tensor_tensor ops (broadcast APs).  A tile straddling two
        windows gets one meta column per window; out-of-window slots carry
        dstl = 999 and vanish in the compare;
      * the TensorEngine computes msg.T @ S per meta column, accumulating
        transposed window aggregates [64, 128] in an fp32 PSUM slab
        [64, GW, 128] (one slice per window of the group) -- a segment-sum
        with no scatter.  Self-loops take a descriptor-free path: the
        window's own rows are DMAd sequentially and hit the same matmul
        with a diagonal one-hot.
  - Layer 1 epilogue: ReLU(agg + b1) on ScalarE, transpose back via PE, store
    h1 rows, AllGather, repeat aggregation for layer 2.
  - Layer 2 epilogue: (agg2 @ W2 + b2) using linearity  A_hat(h1 W2) =
    (A_hat h1) W2, transpose, write dense output rows.
"""

import os
import sys

import numpy as np

sys.path.insert(0, "/opt/trn_rl_repo")

import ml_dtypes
import concourse.bacc as bacc
import concourse.mybir as mybir
import concourse.tile as tile
from concourse.bass_utils import run_bass_kernel_spmd

F32 = mybir.dt.float32
BF16 = mybir.dt.bfloat16
I16 = mybir.dt.int16
NPBF = ml_dtypes.bfloat16

P = 8           # NeuronCores
NOMATCH = 999.0  # dstl value that can never match iota 0..127


class Cfg:
    def __init__(self, n_nodes, f_in=128, f_h=64, f_out=40, nchunk=None):
        assert n_nodes % P == 0
        self.N = n_nodes
        self.SLICE = n_nodes // P
        self.NW = -(-self.SLICE // 128)          # windows per core
        self.PSLICE = self.NW * 128              # padded slice rows
        self.NPAD = P * self.PSLICE              # padded global rows
        if nchunk is None:
            nchunk = max(1, -(-self.NPAD // 32512))
            while self.NPAD % nchunk:
                nchunk += 1
        self.NCHUNK = nchunk
        self.CHROWS = self.NPAD // self.NCHUNK   # rows per gather chunk
        assert self.CHROWS <= 32767, "chunk must fit int16 gather indices"
        for gw in (7, 8, 6, 5, 9, 4, 10, 3, 11, 2, 13, 14, 12, 1):
            if self.NW % gw == 0:
                self.GW = gw                     # windows per gather group
                break
        self.NG = self.NW // self.GW             # gather groups
        self.F_IN, self.F_H, self.F_OUT = f_in, f_h, f_out
        self.FP = 128                            # padded (bf16) feature row


def build_plan(cfg, src, dst, w_edge):
    """Pack non-self edges by (core, dst-window, src-chunk) at shared
    (max-over-cores) edge-count capacities; emit per-core gather indices and
    per-(tile, window-segment) metadata columns."""
    NW, NC, GW, NG = cfg.NW, cfg.NCHUNK, cfg.GW, cfg.NG

    core = dst // cfg.SLICE
    loc = dst - core * cfg.SLICE
    wloc = loc // 128
    dstl = (loc % 128).astype(np.float32)
    gsrc = (src // cfg.SLICE) * cfg.PSLICE + (src % cfg.SLICE)
    chunk = gsrc // cfg.CHROWS
    cidx = (gsrc % cfg.CHROWS).astype(np.int16)

    bucket = (core * NW + wloc) * NC + chunk
    order = np.argsort(bucket, kind="stable")
    counts = np.bincount(bucket, minlength=P * NW * NC).reshape(P, NW, NC)
    cap = counts.max(axis=0)                     # [NW, NC] edge slots, shared

    # static call layout + meta-column enumeration
    off = np.zeros((NW, NC), np.int64)           # slot offset within call
    CT = np.zeros((NG, NC), np.int64)            # slots per call (x128)
    slotbase = np.zeros((NG, NC), np.int64)      # global slot base of call
    # per call: list of (col -> (tile, wi, w)) and per (w,c): col per tile
    calls = {}
    colmap = {}
    ncols = 0
    srun = 0
    col_list = []                                # (g, c, tile, w)
    for g in range(NG):
        for c in range(NC):
            slotbase[g, c] = srun
            t = 0
            spans = []
            for w in range(g * GW, (g + 1) * GW):
                off[w, c] = t
                spans.append((w, t, t + int(cap[w, c])))
                t += int(cap[w, c])
            ct = -(-t // 128) * 128
            CT[g, c] = ct
            srun += ct
            cols = []
            for tt in range(ct // 128):
                lo, hi = tt * 128, tt * 128 + 128
                for (w, a, b) in spans:
                    if a < hi and b > lo:
                        colmap[(w, c, tt)] = ncols
                        cols.append((tt, w))
                        col_list.append((g, c, tt, w))
                        ncols += 1
            calls[(g, c)] = cols
    tot_slots = srun
    tot_cols = ncols

    starts = np.zeros(P * NW * NC + 1, np.int64)
    np.cumsum(counts.ravel(), out=starts[1:])
    rank = np.arange(len(order), dtype=np.int64) - starts[bucket[order]]

    e_core = core[order]
    e_w = wloc[order]
    e_c = chunk[order]
    e_g = e_w // GW
    e_slot = off[e_w, e_c] + rank                # slot within call
    e_tile = e_slot // 128
    e_p = e_slot % 128
    cmap = np.full((NW, NC, int(cap.max()) // 128 + 2), -1, np.int64)
    for (w, c, tt), j in colmap.items():
        ft = off[w, c] // 128
        if 0 <= tt - ft < cmap.shape[2]:
            cmap[w, c, tt - ft] = j
    e_col = cmap[e_w, e_c, e_tile - off[e_w, e_c] // 128]
    assert (e_col >= 0).all()

    dstl_arr = np.full((P, 128, tot_cols + NW), NOMATCH, np.float32)
    wv_arr = np.zeros((P, 128, tot_cols + NW), np.float32)
    dstl_arr[e_core, e_p, e_col] = dstl[order]
    wv_arr[e_core, e_p, e_col] = w_edge[order]

    gslot = slotbase[e_g, e_c] + e_slot          # global slot
    icol = gslot // 16
    ipart = gslot % 16
    idx_arr = np.zeros((P, 16, tot_slots // 16), np.int16)
    idx_arr[e_core, ipart, icol] = cidx[order]
    idx_arr = np.tile(idx_arr, (1, 8, 1))        # replicate across partitions

    tables = dict(CT=CT, slotbase=slotbase, calls=calls,
                  tot_slots=int(tot_slots), tot_cols=int(tot_cols))
    return tables, idx_arr, dstl_arr, wv_arr


def build_program(cfg, tables):
    NW, NC, GW, NG = cfg.NW, cfg.NCHUNK, cfg.GW, cfg.NG
    FH, FOUT, PS, FP = cfg.F_H, cfg.F_OUT, cfg.PSLICE, cfg.FP
    CT, slotbase, calls = tables["CT"], tables["slotbase"], tables["calls"]
    TC = tables["tot_cols"]
    TS = tables["tot_slots"]
    NTMAX = [int(CT[:, c].max()) // 128 for c in range(NC)]
    # per-call col base (cols are enumerated call-major)
    colbase = {}
    run = 0
    for g in range(NG):
        for c in range(NC):
            colbase[(g, c)] = run
            run += len(calls[(g, c)])
    NCMAX = [max(len(calls[(g, c)]) for g in range(NG)) for c in range(NC)]

    nc = bacc.Bacc("TRN2", num_devices=P, debug=False)

    x_d = nc.dram_tensor("x", [PS, 128], F32, kind="ExternalInput")
    W1_d = nc.dram_tensor("W1", [128, FH], F32, kind="ExternalInput")
    W2_d = nc.dram_tensor("W2", [FH, FOUT], F32, kind="ExternalInput")
    b1_d = nc.dram_tensor("b1", [FH, 1], F32, kind="ExternalInput")
    b2_d = nc.dram_tensor("b2", [FOUT, 1], F32, kind="ExternalInput")
    io_d = nc.dram_tensor("iota", [128, 128], BF16, kind="ExternalInput")
    id_d = nc.dram_tensor("ident", [128, 128], F32, kind="ExternalInput")
    ix_d = nc.dram_tensor("idx", [128, TS // 16], I16, kind="ExternalInput")
    dl_d = nc.dram_tensor("dstl", [128, TC + NW], BF16, kind="ExternalInput")
    wv_d = nc.dram_tensor("wv", [128, TC + NW], BF16, kind="ExternalInput")
    out_d = nc.dram_tensor("out", [PS, FOUT], F32, kind="ExternalOutput")

    rg = [list(range(P))]

    with tile.TileContext(nc) as tc:
        with (
            tc.tile_pool(name="const", bufs=1) as cpool,
            tc.tile_pool(name="meta", bufs=1) as mpool,
            tc.tile_pool(name="hown", bufs=1) as hpool,
            tc.tile_pool(name="gather", bufs=2) as gpool,
            tc.tile_pool(name="gidx", bufs=2) as ixpool,
            tc.tile_pool(name="xw", bufs=3) as xwpool,
            tc.tile_pool(name="ws", bufs=6) as wpool,
            tc.tile_pool(name="sbuild", bufs=2) as spool,
            tc.tile_pool(name="sself", bufs=2) as sspool,
            tc.tile_pool(name="psA", bufs=3, space="PSUM") as psA,
            tc.tile_pool(name="psB", bufs=2, space="PSUM") as psB,
            tc.tile_pool(name="dram", bufs=1, space="DRAM") as dpool,
        ):
            W1s = cpool.tile([128, FH], F32, tag="W1")
            nc.sync.dma_start(W1s[:], W1_d[:])
            W2s = cpool.tile([FH, FOUT], F32, tag="W2")
            nc.sync.dma_start(W2s[:], W2_d[:])
            b1s = cpool.tile([FH, 1], F32, tag="b1")
            nc.sync.dma_start(b1s[:], b1_d[:])
            b2s = cpool.tile([FOUT, 1], F32, tag="b2")
            nc.sync.dma_start(b2s[:], b2_d[:])
            ios = cpool.tile([128, 128], BF16, tag="iota")
            nc.sync.dma_start(ios[:], io_d[:])
            ids = cpool.tile([128, 128], F32, tag="ident")
            nc.sync.dma_start(ids[:], id_d[:])
            dls = mpool.tile([128, TC + NW], BF16, tag="dstl")
            nc.sync.dma_start(dls[:], dl_d[:])
            wvs = mpool.tile([128, TC + NW], BF16, tag="wv")
            nc.sync.dma_start(wvs[:], wv_d[:])

            # bf16 h tensors with 256B row stride (only cols 0:FH are real)
            h_loc = dpool.tile([PS, FP], BF16, tag="h_loc")
            h1_loc = dpool.tile([PS, FP], BF16, tag="h1_loc")
            h_full = dpool.tile([cfg.NPAD, FP], BF16, tag="h_full", addr_space="Shared")
            h1_full = dpool.tile([cfg.NPAD, FP], BF16, tag="h1_full", addr_space="Shared")

            h_loc_v = h_loc[:].rearrange("(w p) f -> p w f", p=128)
            h1_loc_v = h1_loc[:].rearrange("(w p) f -> p w f", p=128)
            x_v = x_d[:].rearrange("(w p) f -> p w f", p=128)
            out_v = out_d[:].rearrange("(w p) f -> p w f", p=128)

            def build_S(pool, tag, ncols_t, meta_lo, ncols):
                """Two wide DVE ops -> S[:, j, n] = wv_j * (iota_n == dstl_j)."""
                S = pool.tile([128, ncols_t, 128], BF16, tag=tag)
                io_b = ios[:].unsqueeze(1).broadcast_to([128, ncols, 128])
                dl_b = dls[:, meta_lo:meta_lo + ncols].unsqueeze(2) \
                    .broadcast_to([128, ncols, 128])
                wv_b = wvs[:, meta_lo:meta_lo + ncols].unsqueeze(2) \
                    .broadcast_to([128, ncols, 128])
                nc.vector.tensor_tensor(S[:, :ncols, :], io_b, dl_b,
                                        mybir.AluOpType.is_equal)
                nc.vector.tensor_tensor(S[:, :ncols, :], S[:, :ncols, :], wv_b,
                                        mybir.AluOpType.mult)
                return S

            # ---- Phase A: h = x @ W1, stored row-major bf16 ---------------
            for w in range(NW):
                xw = xwpool.tile([128, 128], F32, tag="xw")
                nc.sync.dma_start(xw[:], x_v[:, w, :])
                pxT = psB.tile([128, 128], F32, tag="row")
                nc.tensor.matmul(pxT[:], xw[:], ids[:], start=True, stop=True)
                xT = xwpool.tile([128, 128], F32, tag="xT")
                nc.scalar.copy(xT[:], pxT[:])
                pa = psA.tile([FH, GW, 128], F32, tag="agg")
                nc.tensor.matmul(pa[:, 0, :], W1s[:], xT[:],
                                 start=True, stop=True)
                hT = wpool.tile([FH, 128], F32, tag="hT")
                nc.scalar.copy(hT[:], pa[:, 0, :])
                pb = psB.tile([128, 128], F32, tag="row")
                nc.tensor.matmul(pb[:, 0:FH], hT[:], ids[0:FH, 0:FH],
                                 start=True, stop=True)
                hr = wpool.tile([128, FH], BF16, tag="hrow")
                nc.scalar.copy(hr[:], pb[:, 0:FH])
                nc.sync.dma_start(h_loc_v[:, w, 0:FH], hr[:])

            nc.gpsimd.collective_compute(
                "AllGather", mybir.AluOpType.bypass, replica_groups=rg,
                ins=[h_loc[:].opt()], outs=[h_full[:].opt()],
            )

            # ---- shared aggregation layer --------------------------------
            def agg_layer(src_full, loc_v, last):
                hown = hpool.tile([128, NW, FH], BF16,
                                  tag="hown2" if last else "hown1")
                nc.sync.dma_start(hown[:], loc_v[:, :, 0:FH])
                for g in range(NG):
                    gts, Ss = [], []
                    for c in range(NC):
                        ct = int(CT[g, c])
                        nt = ct // 128
                        gt = gpool.tile([128, NTMAX[c], FP], BF16, tag=f"g{c}")
                        gts.append(gt)
                        ncol = len(calls[(g, c)])
                        if ct == 0:
                            Ss.append(None)
                            continue
                        ixt = ixpool.tile([128, NTMAX[c] * 8], I16, tag=f"i{c}")
                        nc.sync.dma_start(
                            ixt[:, : ct // 16],
                            ix_d[:, slotbase[g, c] // 16:
                                 (slotbase[g, c] + ct) // 16],
                        )
                        nc.gpsimd.dma_gather(
                            out_ap=gt[:, :nt, :],
                            in_ap=src_full[c * cfg.CHROWS:(c + 1) * cfg.CHROWS, :],
                            idxs_ap=ixt[:, : ct // 16],
                            num_idxs=ct,
                            num_idxs_reg=ct,
                            elem_size=FP,
                            single_packet=False,
                        )
                        Ss.append(build_S(spool, f"S{c}", NCMAX[c],
                                          colbase[(g, c)], ncol))
                    Sself = build_S(sspool, "Sself", GW, TC + g * GW, GW)
                    pw = psA.tile([FH, GW, 128], F32, tag="agg")
                    nc.vector.memset(pw[:], 0.0)
                    for c in range(NC):
                        for ci, (tt, w) in enumerate(calls[(g, c)]):
                            wi = w - g * GW
                            nc.tensor.matmul(
                                pw[:, wi, :],
                                gts[c][:, tt, 0:FH],
                                Ss[c][:, ci, :],
                                start=False, stop=False,
                                skip_group_check=True,
                            )
                    for wi in range(GW):
                        w = g * GW + wi
                        # self-loop tile: diagonal one-hot over own rows
                        nc.tensor.matmul(pw[:, wi, :], hown[:, w, :],
                                         Sself[:, wi, :],
                                         start=False, stop=True,
                                         skip_group_check=True)
                        if not last:
                            hT1 = wpool.tile([FH, 128], F32, tag="hT")
                            nc.scalar.activation(
                                hT1[:], pw[:, wi, :],
                                mybir.ActivationFunctionType.Relu,
                                bias=b1s[:, 0:1],
                            )
                            pb = psB.tile([128, 128], F32, tag="row")
                            nc.tensor.matmul(pb[:, 0:FH], hT1[:],
                                             ids[0:FH, 0:FH],
                                             start=True, stop=True)
                            hr = wpool.tile([128, FH], BF16, tag="hrow")
                            nc.scalar.copy(hr[:], pb[:, 0:FH])
                            nc.sync.dma_start(h1_loc_v[:, w, 0:FH], hr[:])
                        else:
                            a2 = wpool.tile([FH, 128], F32, tag="hT")
                            nc.scalar.copy(a2[:], pw[:, wi, :])
                            po = psB.tile([128, 128], F32, tag="row")
                            nc.tensor.matmul(po[0:FOUT, :], W2s[:], a2[:],
                                             start=True, stop=True)
                            o2T = wpool.tile([FOUT, 128], F32, tag="o2T")
                            nc.vector.tensor_scalar(
                                o2T[:], po[0:FOUT, :], b2s[:, 0:1], None,
                                mybir.AluOpType.add,
                            )
                            pf = psB.tile([128, 128], F32, tag="row")
                            nc.tensor.matmul(pf[:, 0:FOUT], o2T[:],
                                             ids[0:FOUT, 0:FOUT],
                                             start=True, stop=True)
                            orow = wpool.tile([128, FOUT], F32, tag="orow")
                            nc.scalar.copy(orow[:], pf[:, 0:FOUT])
                            nc.sync.dma_start(out_v[:, w, :], orow[:])

            agg_layer(h_full[:], h_loc_v, last=False)
            nc.gpsimd.collective_compute(
                "AllGather", mybir.AluOpType.bypass, replica_groups=rg,
                ins=[h1_loc[:].opt()], outs=[h1_full[:].opt()],
            )
            agg_layer(h1_full[:], h1_loc_v, last=True)

    nc.compile()
    return nc


def _prep(cfg, x, edge_index, W1, b1, W2, b2):
    src = np.asarray(edge_index[0], dtype=np.int64)
    dst = np.asarray(edge_index[1], dtype=np.int64)
    x = np.asarray(x, dtype=np.float32)
    N = cfg.N

    deg = (np.bincount(dst, minlength=N) + 1.0).astype(np.float32)
    dinv = (1.0 / np.sqrt(deg)).astype(np.float32)

    w_e = (dinv[src] * dinv[dst]).astype(np.float32)
    tables, idx_arr, dstl_arr, wv_arr = build_plan(cfg, src, dst, w_e)

    # self-loop meta (the NW trailing columns): diagonal one-hot weights
    NW = cfg.NW
    TC = tables["tot_cols"]
    selfw = np.zeros((P, 128, NW), np.float32)
    d2 = (dinv * dinv).reshape(P, cfg.SLICE)
    for k in range(P):
        flat = np.zeros(cfg.PSLICE, np.float32)
        flat[: cfg.SLICE] = d2[k]
        selfw[k] = flat.reshape(NW, 128).T
    dstl_arr[:, :, TC:] = np.broadcast_to(
        np.arange(128, dtype=np.float32)[None, :, None], (P, 128, NW))
    wv_arr[:, :, TC:] = selfw

    # per-core x slices, zero-padded to PSLICE rows
    xs = np.zeros((P, cfg.PSLICE, cfg.F_IN), np.float32)
    xs[:, : cfg.SLICE] = x.reshape(P, cfg.SLICE, cfg.F_IN)

    iota = np.broadcast_to(np.arange(128, dtype=np.float32),
                           (128, 128)).astype(NPBF)
    ident = np.eye(128, dtype=np.float32)
    W1f = np.asarray(W1, np.float32)
    W2f = np.asarray(W2, np.float32)
    b1f = np.asarray(b1, np.float32).reshape(cfg.F_H, 1)
    b2f = np.asarray(b2, np.float32).reshape(cfg.F_OUT, 1)

    in_maps = []
    for k in range(P):
        in_maps.append({
            "x": xs[k], "W1": W1f, "W2": W2f, "b1": b1f, "b2": b2f,
            "iota": iota, "ident": ident,
            "idx": idx_arr[k],
            "dstl": dstl_arr[k].astype(NPBF),
            "wv": wv_arr[k].astype(NPBF),
        })
    return tables, in_maps


def _enable_tracing():
    """This container's antenv lacks axon_hooks; install the NTFF profile
    hook ourselves and stub out the S3 artifact upload."""
    import types
    import antenv
    import concourse.bass_utils as bu

    if "antenv.axon_hooks" not in sys.modules:
        from trn_agent_boot.trn_boot import _ntff_profile_via_ctypes

        hook = _ntff_profile_via_ctypes("/opt/axon/libaxon_pjrt.so")
        mod = types.ModuleType("antenv.axon_hooks")
        mod.get_axon_ntff_profile_hook = lambda: hook
        sys.modules["antenv.axon_hooks"] = mod
        antenv.axon_hooks = mod
    bu.upload_artifacts = lambda tmpdir: "local://" + tmpdir


def run(cfg, x, edge_index, W1, b1, W2, b2, trace=False):
    tables, in_maps = _prep(cfg, x, edge_index, W1, b1, W2, b2)
    nc = build_program(cfg, tables)
    if trace:
        try:
            _enable_tracing()
        except Exception as e:  # tracing is best-effort
            print("tracing unavailable:", e)
            trace = False
    res = run_bass_kernel_spmd(nc, in_maps, core_ids=list(range(P)), trace=trace)
    outs = [res.results[k]["out"][: cfg.SLICE] for k in range(P)]
    out = np.concatenate(outs, axis=0)
    return out, res


def kernel(x, edge_index, W1, b1, W2, b2):
    cfg = Cfg(n_nodes=100000, f_in=128, f_h=64, f_out=40)
    trace = bool(os.environ.get("BASS_TRACE"))
    out, res = run(cfg, x, edge_index, W1, b1, W2, b2, trace=trace)
    if res.exec_time_ns is not None:
        print(f"HW exec time: {res.exec_time_ns} ns")
    return out.astype(np.float32)
